# revision 83
# baseline (speedup 1.0000x reference)
"""DeepseekV3 MLA forward on 8 TRN2 NeuronCores.

Sharding: data-parallel over batch (B=2 -> 2 groups of 4 cores), tensor-
parallel over heads within each batch group (32 heads -> 4 groups of 8).

vs the original replicated-latents kernel (592us -> 339us):
  * RMSNorm weights folded into wq_b / wkv_b rows on the host; the device
    norm is x * rsqrt(mean(x^2)+eps), and the kv-path rstd scaling commutes
    into stage D's output copies so D starts on raw latents.
  * The q-latent projection (45% of the old FLOPs) is token-split across
    the 4 cores of each batch group: each core projects+norms its 256-token
    quarter, then a 4-core HBM AllGather (fp8, 0.39MB in / 1.6MB out per
    core) distributes the normalized q-latents. The kv path stays fully
    replicated: its compute hides the collective completely.
  * Every weight*activation GEMM runs as fp8-e4m3 DoubleRow chains with
    exact hi + lo/256 weight splitting (more accurate than bf16 weights,
    0.5 cycles/row): stages A-q/A-kv/F split both operands (3 chains, 75%
    of bf16 cost), stage B rides the already-fp8 gathered latents (2
    chains, 50%). Attention scores/av stay bf16; PSUM is always f32.
  * Causal trim: diagonal score chunks skip fully-masked query columns in
    scores/exp/denominator/av; the remaining triangular band mask is a
    single 128-col cmask window.
  * Software-pipelined attention (scores of unit i+3 ahead of pd/av of
    unit i), E(1),E(0),F(1),F(0) phase order with the attention fp8
    conversions hidden under F(1), nt-outer F loop so wo loads once,
    f16 output tensor (host upcasts).
  * Measured on HW: rel err 1.38e-2 (gate 2e-2), 336976 ns per core.

Dataflow on device keeps activations transposed ([feature, token]) so
every matmul contracts over the partition dim with no on-device
transposes anywhere:
  qlatT   = wq_a.T @ xT  (local 256 tokens, chunk-major over rank)
  kvlatT  = wkv_a.T @ xT (all tokens, quart-major over HID)
  qT_h    = wq_b_h.T @ qlatT                                   [d, T]
  scoresT = kT_h-chunks @ qT_h                                 [tk, tq]
  softmax over tk (=partitions): exp on ACT, denominator via a
  ones[128,128] matmul (result replicated across partitions), then
  attn_outT = v_chunks.T @ expT                                [dv, tq]
  out      = attnT-chunks.T @ wo_h  (natural layout)           [tq, hid]
RoPE in transposed layout: rot(x) = x*cos + swap32(x)*(+-sin), where
swap32 exchanges the two 32-row halves of each 64-row rope block (done
with SBUF->SBUF block DMAs) and the +-sin sign pattern is host-built.
"""

import os
import sys

import numpy as np

sys.path.insert(0, "/opt/trn_rl_repo")

B, T, HID = 2, 1024, 4096
H, D_NOPE, D_ROPE, D_V = 32, 128, 64, 128
D_QK = D_NOPE + D_ROPE
Q_RANK, KV_RANK = 1536, 512
THETA, EPS = 10000.0, 1e-6
SCALE = float(D_QK) ** -0.5
NMASK = -30000.0

HG = H // 4          # heads per core = 8
P = 128
QCH = Q_RANK // P    # 12 latent chunks (q)
KCH = KV_RANK // P   # 4 latent chunks (kv)
HIDK = HID // P      # 32 contraction tiles
KQ = HIDK // 4       # 8 k-tiles per quart
TQ = 512             # token tile (free dim) for most matmuls
NT = T // TQ         # 2 token tiles
TC = T // P          # 8 token chunks of 128
NHID = HID // TQ     # 8 output column tiles
TL = T // 4          # 256 local tokens per core (q path token split)

_CACHED = {}
STAGE_MARKS = []


def _build_program():
    import contextlib

    import concourse.bacc as bacc
    import concourse.mybir as mybir
    import concourse.tile as tile

    f32 = mybir.dt.float32
    bf16 = mybir.dt.bfloat16
    AF = mybir.ActivationFunctionType
    ALU = mybir.AluOpType
    DR = mybir.MatmulPerfMode.DoubleRow

    nc = bacc.Bacc()
    del STAGE_MARKS[:]

    # ---- DRAM I/O (per-core; SPMD across the 8 cores) ----
    # leading dim 2 = fp8 hi / (residual*256) lo pair
    f8_ = mybir.dt.float8e4
    xloc = nc.dram_tensor("xloc", (2, P, HIDK, TL), f8_, kind="ExternalInput")
    xT = nc.dram_tensor(
        "xT", (2, HIDK // 2, P, 2, T), f8_, kind="ExternalInput")
    wqac = nc.dram_tensor(
        "wqac", (QCH, P, 2, HIDK, P), f8_, kind="ExternalInput")
    wkva = nc.dram_tensor(
        "wkva", (2, 2, 4, P, KQ, 256), f8_, kind="ExternalInput")
    wkvr = nc.dram_tensor("wkvr", (2, 4, P, KQ, 64), f8_, kind="ExternalInput")
    f8i = mybir.dt.float8e4
    wqbn = nc.dram_tensor("wqbn", (2, HG, P, QCH, P), f8i, kind="ExternalInput")
    wqbr = nc.dram_tensor(
        "wqbr", (2, HG // 2, P, QCH, P), f8i, kind="ExternalInput")
    wkbn = nc.dram_tensor("wkbn", (P, HG, KCH, P), bf16, kind="ExternalInput")
    wkbv = nc.dram_tensor("wkbv", (2, P, KCH, TQ), bf16, kind="ExternalInput")
    wo = nc.dram_tensor(
        "wo", (2, NHID, 2, P, 4, TQ), mybir.dt.float8e4, kind="ExternalInput")
    cos4 = nc.dram_tensor("cos4", (P, T), bf16, kind="ExternalInput")
    sin4 = nc.dram_tensor("sin4", (P, T), bf16, kind="ExternalInput")  # +-sin
    cmask = nc.dram_tensor("cmask", (P, 7 * P), f32, kind="ExternalInput")
    kbias = nc.dram_tensor("kbias", (P, TC), f32, kind="ExternalInput")
    onesd = nc.dram_tensor("onesd", (P, P), bf16, kind="ExternalInput")
    f8 = mybir.dt.float8e4
    gin = nc.dram_tensor("gin", (P, QCH, TL), f8, kind="Internal")
    gout = nc.dram_tensor("gout", (4, P, QCH, TL), f8, kind="Internal")
    f16 = mybir.dt.float16
    out = nc.dram_tensor("out", (NT, NHID, 4, P, TQ), f16, kind="ExternalOutput")

    with tile.TileContext(nc) as tc, contextlib.ExitStack() as rstack:
        with (
            tc.tile_pool(name="const", bufs=1) as const,
            tc.tile_pool(name="psmm", bufs=3, space="PSUM") as psum,
            tc.tile_pool(name="pspd", bufs=2, space="PSUM") as pspd,
            tc.tile_pool(name="pssc", bufs=3, space="PSUM") as pssc,
        ):
            ones_sb = const.tile([P, P], bf16, tag="ones")
            cos_sb = const.tile([P, T], bf16, tag="cos")
            sin_sb = const.tile([P, T], bf16, tag="sin")
            kb_sb = const.tile([P, TC], f32, tag="kb")
            zero_b = const.tile([P, 1], f32, tag="zb")
            nc.vector.memset(zero_b[:], 0.0)
            eps_b = const.tile([P, 1], f32, tag="eb")
            nc.vector.memset(eps_b[:], EPS)

            def emit_const_dmas():
                # ones feeds the A-q sum-of-squares matmuls (~12us in);
                # cos/sin/kb aren't read until the kv rope / EF exp, so they
                # ride behind the quart-1 x stream instead of delaying the
                # compute-gating wqac pieces at the head of the DMA queue
                nc.sync.dma_start(ones_sb[:], onesd[:, :])

            def emit_late_const_dmas():
                nc.sync.dma_start(cos_sb[:], cos4[:, :])
                nc.sync.dma_start(sin_sb[:], sin4[:, :])
                nc.sync.dma_start(kb_sb[:], kbias[:, :])

            # long-lived activations (right side of SBUF)
            qlatp = rstack.enter_context(
                tc.tile_pool(name="qlatp", bufs=1, side="right"))
            qlat_all = qlatp.tile([P, QCH, T], f8, tag="qlat")
            kpep = rstack.enter_context(
                tc.tile_pool(name="kpep", bufs=1, side="right"))
            kpe2 = kpep.tile([P, T], bf16, tag="kpe2")
            kTnp = rstack.enter_context(
                tc.tile_pool(name="kTnp", bufs=1, side="right"))
            kTn = kTnp.tile([P, HG, T], bf16, tag="kTn")
            vqp = rstack.enter_context(
                tc.tile_pool(name="vqp", bufs=2, side="right"))
            vq = [
                vqp.tile([P, TC, 4 * D_V], bf16, tag="vq", name="vq")
                for _ in range(2)
            ]

            wqb0p = rstack.enter_context(
                tc.tile_pool(name="wqb0p", bufs=1, side="right"))
            wqb0 = (
                wqb0p.tile([P, QCH, P], f8, tag="wqb0h", name="wqb0h"),
                wqb0p.tile([P, QCH, P], f8, tag="wqb0l", name="wqb0l"),
            )

            qtmpp = rstack.enter_context(
                tc.tile_pool(name="qtmpp", bufs=1, side="right"))

            # kv-path x tiles: pool opened early so quart 0/1 loads can be
            # issued while stage A-q still computes (SP queue is in-order);
            # closed right after stage D to free SBUF for the EF stage
            xk_stack = contextlib.ExitStack()
            xkp = xk_stack.enter_context(
                tc.tile_pool(name="xk", bufs=12, side="right"))
            wAp = xk_stack.enter_context(
                tc.tile_pool(name="wA", bufs=2, side="right"))

            def load_xk(quart):
                # paired tiles [P, 2, T] so DoubleRow sees adjacent k-tiles
                tiles = []
                for k2 in range(KQ // 2):
                    kp = quart * (KQ // 2) + k2
                    xh_ = xkp.tile([P, 2, T], f8, tag="xkh", name="xh_")
                    nc.sync.dma_start(xh_[:], xT[0, kp, :, :, :])
                    xl_ = xkp.tile([P, 2, T], f8, tag="xkl", name="xl_")
                    nc.sync.dma_start(xl_[:], xT[1, kp, :, :, :])
                    tiles.append((xh_, xl_))
                return tiles

            kvblocks = [
                (wkva, 0, 256, 0),
                (wkva, 1, 256, 2),
                (wkvr, None, 64, 4),
            ]

            def emit_wA(quart):
                # tiles sized to the block width so every DMA lands in
                # contiguous >=512B runs (a [:, :, :64] slice of a 256-wide
                # tile would write 64B strided runs at 2x latency)
                wts = []
                for bi, (wdram, blki, width, dch) in enumerate(kvblocks):
                    pair = []
                    for hl_ in range(2):
                        wt = wAp.tile(
                            [P, KQ, width], f8, tag=f"wA{bi}{hl_}", name="wt")
                        wsrc = (
                            wdram[hl_, quart, :, :, :]
                            if blki is None
                            else wdram[hl_, blki, quart, :, :, :]
                        )
                        nc.sync.dma_start(wt[:], wsrc)
                        pair.append(wt)
                    wts.append(pair)
                return wts

            # ---- stage A-q: local-quarter q latents, chunk-major ----
            with (
                tc.tile_pool(name="xlocp", bufs=1) as xlocp,
                tc.tile_pool(name="wqap", bufs=3) as wqap,

                tc.tile_pool(name="wrkq", bufs=3) as wrkq,
            ):
                # PE p-state warm-up: dummy matmuls on a memset tile while
                # the first real DMAs land, so real matmuls start at full
                # clock instead of paying the 3us ramp
                warm = wrkq.tile([P, TQ], bf16, tag="warm", name="warm")
                nc.vector.memset(warm[:], 0.0)
                wps = pssc.tile([P, TQ], f32, tag="psc", name="wps")
                for _ in range(26):
                    nc.tensor.matmul(
                        wps[:], warm[:, :P], warm[:], start=True, stop=True)

                xh_sb = xlocp.tile([P, HIDK, TL], f8, tag="xlh")
                nc.sync.dma_start(xh_sb[:], xloc[0, :, :, :])
                xl_sb = xlocp.tile([P, HIDK, TL], f8, tag="xll")
                qtmp = qtmpp.tile([P, QCH, TL], bf16, tag="qtmp")
                # fp8 staging for the gather: quantized once, from the
                # normalized bf16 values (scores-only path; validated at
                # 1.4e-2 max-rel-err vs the 2e-2 gate)
                qgin = qtmpp.tile([P, QCH, TL], f8, tag="qgin")
                ssq = pspd.tile([P, TQ], f32, tag="pd", name="ssq")
                prev_sq = None
                for m in range(QCH):
                    whl = wqap.tile([P, 2, HIDK, P], f8, tag="wqa", name="whl")
                    nc.sync.dma_start(whl[:], wqac[m, :, :, :, :])
                    if m == 0:
                        nc.sync.dma_start(xl_sb[:], xloc[1, :, :, :])
                        emit_const_dmas()
                    if m == 8:
                        # stage A-kv quart-0 x tiles prefetched behind the
                        # remaining (compute-gated) wqac loads, so the kv
                        # path starts the moment A-q's matmuls finish
                        xk_pre = [load_xk(0)]
                    ps = psum.tile([P, TQ], f32, tag="mm", name="psq")
                    psl = pssc.tile([P, TQ], f32, tag="psc", name="psql")
                    # main chain first: the mid chains' xl operand lands
                    # a few us after xh, so interleaving stalls chunk 0
                    for k2 in range(HIDK // 2):
                        sl2 = slice(2 * k2, 2 * k2 + 2)
                        nc.tensor.matmul(
                            ps[:, :TL], whl[:, 0, sl2, :], xh_sb[:, sl2, :],
                            start=(k2 == 0), stop=(k2 == HIDK // 2 - 1),
                            perf_mode=DR)
                    for k2 in range(HIDK // 2):
                        sl2 = slice(2 * k2, 2 * k2 + 2)
                        nc.tensor.matmul(
                            psl[:, :TL], whl[:, 0, sl2, :], xl_sb[:, sl2, :],
                            start=(k2 == 0), stop=False, perf_mode=DR)
                        nc.tensor.matmul(
                            psl[:, :TL], whl[:, 1, sl2, :], xh_sb[:, sl2, :],
                            start=False, stop=(k2 == HIDK // 2 - 1),
                            perf_mode=DR)
                    if prev_sq is not None:
                        nc.tensor.matmul(
                            ssq[:, :TL],
                            ones_sb[:],
                            prev_sq[:],
                            start=(m == 1),
                            stop=False,
                        )
                    mid_sb = wrkq.tile([P, TL], bf16, tag="mid", name="mid_sb")
                    nc.scalar.mul(mid_sb[:], psl[:, :TL], 1.0 / 256.0)
                    nc.vector.tensor_tensor(
                        qtmp[:, m, :], mid_sb[:], ps[:, :TL], ALU.add)
                    sq = wrkq.tile([P, TL], bf16, tag="sq", name="sq")
                    nc.scalar.activation(
                        sq[:], qtmp[:, m, :], AF.Square, bias=zero_b[:])
                    prev_sq = sq
                nc.tensor.matmul(
                    ssq[:, :TL], ones_sb[:], prev_sq[:], start=False, stop=True)
                # quart-0 kv weights + second x quart: issued right after
                # the last compute-gated wqac load
                wA_pre = emit_wA(0)
                xk_pre.append(load_xk(1))
                emit_late_const_dmas()
                std = wrkq.tile([P, TL], f32, tag="std", name="std")
                nc.scalar.activation(
                    std[:], ssq[:, :TL], AF.Sqrt, bias=eps_b[:],
                    scale=1.0 / Q_RANK)
                rstd = qtmpp.tile([P, TL], f32, tag="rstd", name="rstd")
                with nc.allow_low_precision("rmsnorm rstd"):
                    nc.vector.reciprocal(rstd[:], std[:])

                def emit_qnorm_gather():
                    # emitted after A-kv quart 1: on the DVE queue these
                    # mults wait on rstd, and emitted any earlier they stall
                    # the quart combines (and the PSUM rotation) behind them
                    for m in range(QCH):
                        nc.vector.tensor_tensor(
                            qgin[:, m, :], qtmp[:, m, :], rstd[:], ALU.mult)
                    nc.sync.dma_start(gin[:, :, :], qgin[:, :, :])
                    nc.gpsimd.collective_compute(
                        "AllGather",
                        mybir.AluOpType.bypass,
                        replica_groups=[[0, 1, 2, 3], [4, 5, 6, 7]],
                        ins=[gin[:, :, :]],
                        outs=[gout[:, :, :, :]],
                    )

            STAGE_MARKS.append(("Akv", nc.next_id()))
            # ---- stage A-kv: full-token kv latents, quart-major ----
            with (
                tc.tile_pool(name="kvlatp", bufs=1) as kvlatp,
                tc.tile_pool(name="kvnp", bufs=1) as kvnp,
                tc.tile_pool(name="rstdp", bufs=2) as rstdp,
                tc.tile_pool(name="wkbnp", bufs=1) as wkbnp,
                tc.tile_pool(name="wkbvp", bufs=1) as wkbvp,
                tc.tile_pool(name="wrkA", bufs=3) as wrkA,
            ):
                kvlat = [
                    kvlatp.tile([P, T], bf16, tag=f"kvlat{i}", name=f"kvlat{i}")
                    for i in range(KCH + 1)
                ]
                kvn = [
                    kvnp.tile([P, T], bf16, tag=f"kvn{i}", name=f"kvn{i}")
                    for i in range(KCH)
                ]
                wkn = wkbnp.tile([P, HG, KCH, P], bf16, tag="wkbn", name="wkbn")
                wkvts = [
                    wkbvp.tile([P, KCH, TQ], bf16, tag=f"wkbv{q_}", name="wkbv")
                    for q_ in range(2)
                ]
                def emit_dweight_dmas():
                    nc.sync.dma_start(wkn[:], wkbn[:, :, :, :])
                    for q_ in range(2):
                        nc.sync.dma_start(wkvts[q_][:], wkbv[q_, :, :, :])
                    nc.sync.dma_start(wqb0[0][:], wqbn[0, 0, :, :, :])
                    nc.sync.dma_start(wqb0[1][:], wqbn[1, 0, :, :, :])

                if True:
                    for quart in range(4):
                        xk = xk_pre[quart] if quart < 2 else load_xk(quart)
                        wts_q = wA_pre if quart == 0 else emit_wA(quart)
                        for mbi, (wdram, blki, width, dch) in \
                                enumerate(kvblocks):
                            nm = (width + P - 1) // P
                            wth, wtl = wts_q[mbi]
                            for m in range(nm):
                                w = min(width - m * P, P)
                                msl = slice(m * P, m * P + w)
                                for t in range(NT):
                                    tsl = slice(t * TQ, (t + 1) * TQ)
                                    psm = psum.tile(
                                        [P, TQ], f32, tag="mm", name="psAm")
                                    psl = pssc.tile(
                                        [P, TQ], f32, tag="psc", name="psAl")
                                    for k2 in range(KQ // 2):
                                        s2 = slice(2 * k2, 2 * k2 + 2)
                                        xh2, xl2 = xk[k2]
                                        nc.tensor.matmul(
                                            psm[:w, :],
                                            wth[:, s2, msl],
                                            xh2[:, :, tsl],
                                            start=(k2 == 0),
                                            stop=(k2 == KQ // 2 - 1),
                                            perf_mode=DR,
                                        )
                                        nc.tensor.matmul(
                                            psl[:w, :],
                                            wth[:, s2, msl],
                                            xl2[:, :, tsl],
                                            start=(k2 == 0),
                                            stop=False,
                                            perf_mode=DR,
                                        )
                                        nc.tensor.matmul(
                                            psl[:w, :],
                                            wtl[:, s2, msl],
                                            xh2[:, :, tsl],
                                            start=False,
                                            stop=(k2 == KQ // 2 - 1),
                                            perf_mode=DR,
                                        )
                                    dst = kvlat[dch + m][:w, tsl]
                                    mids = wrkA.tile(
                                        [P, TQ], bf16, tag="mids", name="mids")
                                    nc.scalar.mul(
                                        mids[:w, :], psl[:w, :], 1.0 / 256.0)
                                    if quart == 0:
                                        nc.vector.tensor_tensor(
                                            dst, mids[:w, :], psm[:w, :],
                                            ALU.add)
                                    else:
                                        nc.gpsimd.tensor_tensor(
                                            dst, dst, mids[:w, :], ALU.add)
                                        nc.vector.tensor_tensor(
                                            dst, psm[:w, :], dst, ALU.add)
                        if quart == 1:
                            emit_qnorm_gather()


                    emit_dweight_dmas()
                    STAGE_MARKS.append(("Anorm", nc.next_id()))
                    # ---- kv rmsnorm (w folded on host): rstd only; the
                    # normalize is folded into stage D's output copies (kT)
                    # and into separate kvn tiles (v path), so D's matmuls
                    # start immediately on the raw latents ----
                    rstdvs = []
                    for t in range(NT):
                        ssp = pspd.tile([P, TQ], f32, tag="pd", name="ssp")
                        for m in range(KCH):
                            sq = wrkA.tile([P, TQ], bf16, tag="sqv", name="sqv")
                            nc.scalar.activation(
                                sq[:],
                                kvlat[m][:, t * TQ : (t + 1) * TQ],
                                AF.Square,
                                bias=zero_b[:],
                            )
                            nc.tensor.matmul(
                                ssp[:],
                                ones_sb[:],
                                sq[:],
                                start=(m == 0),
                                stop=(m == KCH - 1),
                            )
                        stdv = wrkA.tile([P, TQ], f32, tag="stdv", name="stdv")
                        nc.scalar.activation(
                            stdv[:], ssp[:], AF.Sqrt, bias=eps_b[:],
                            scale=1.0 / KV_RANK)
                        rstdv = rstdp.tile(
                            [P, TQ], f32, tag="rstdv", name="rstdv")
                        with nc.allow_low_precision("rmsnorm rstd"):
                            nc.vector.reciprocal(rstdv[:], stdv[:])
                        rstdvs.append(rstdv)
                    ksw = xkp.tile([P, T], bf16, tag="xk", name="ksw")

                STAGE_MARKS.append(("D", nc.next_id()))

                # ---- stage D: kT_nope per head (raw latents, rstd-scaled
                # on the way out), v per quad (normalized kvn copies) ----
                def emit_dkt(h):
                    pp, ptag = (psum, "mm") if h % 2 == 0 else (pssc, "psc")
                    pst = [
                        pp.tile([P, TQ], f32, tag=ptag, name="psD")
                        for _ in range(NT)
                    ]
                    for k in range(KCH):
                        for t in range(NT):
                            nc.tensor.matmul(
                                pst[t][:],
                                wkn[:, h, k, :],
                                kvlat[k][:, t * TQ : (t + 1) * TQ],
                                start=(k == 0),
                                stop=(k == KCH - 1),
                            )
                    for t in range(NT):
                        nc.vector.tensor_tensor(
                            kTn[:, h, t * TQ : (t + 1) * TQ], pst[t][:],
                            rstdvs[t][:], ALU.mult)

                for h in range(4):
                    emit_dkt(h)
                # kv normalize: DVE work emitted here so it rides behind the
                # first kT copies and finishes just before stage D's v part
                for m in range(KCH):
                    for t in range(NT):
                        nc.vector.tensor_tensor(
                            kvn[m][:, t * TQ : (t + 1) * TQ],
                            kvlat[m][:, t * TQ : (t + 1) * TQ],
                            rstdvs[t][:], ALU.mult)
                for h in range(4, HG):
                    emit_dkt(h)
                # k rope (needed first by stage EF - emit late on DVE)
                nc.sync.dma_start(kpe2[0:64, :], kvlat[KCH][0:64, :])
                nc.sync.dma_start(kpe2[64:128, :], kvlat[KCH][0:64, :])
                nc.sync.dma_start(ksw[0:32, :], kvlat[KCH][32:64, :])
                nc.sync.dma_start(ksw[32:64, :], kvlat[KCH][0:32, :])
                nc.sync.dma_start(ksw[64:96, :], kvlat[KCH][32:64, :])
                nc.sync.dma_start(ksw[96:128, :], kvlat[KCH][0:32, :])
                nc.vector.tensor_tensor(ksw[:], ksw[:], sin_sb[:], ALU.mult)
                nc.vector.tensor_tensor(
                    kpe2[:], kpe2[:], cos_sb[:], ALU.mult)
                rope_done = nc.vector.tensor_tensor(
                    kpe2[:], kpe2[:], ksw[:], ALU.add)
                for quad in range(2):
                    for tkc in range(TC):
                        pp, ptag = (
                            (psum, "mm") if tkc % 2 == 0 else (pssc, "psc"))
                        ps_ = pp.tile([P, TQ], f32, tag=ptag, name="psV")
                        for k in range(KCH):
                            nc.tensor.matmul(
                                ps_[:],
                                kvn[k][:, tkc * P : (tkc + 1) * P],
                                wkvts[quad][:, k, :],
                                start=(k == 0),
                                stop=(k == KCH - 1),
                            )
                        nc.vector.tensor_copy(vq[quad][:, tkc, :], ps_[:])

                # gathered q latents -> qlat_all (waits on the collective).
                # Explicit scheduling dep on the k-rope: without it the tile
                # scheduler hoists these ready-early-but-long-wait DMAs ahead
                # of the not-yet-ready kv weight loads on the in-order SP
                # queue, stalling stage A-kv behind the collective.
                from concourse.tile import add_dep_helper
                for g in range(4):
                    gd = nc.sync.dma_start(
                        qlat_all[:, :, g * TL : (g + 1) * TL],
                        gout[g, :, :, :],
                    )
                    add_dep_helper(
                        gd.ins, rope_done.ins, reason="gout after kv path")

            xk_stack.close()
            STAGE_MARKS.append(("B", nc.next_id()))
            # kvlat + stage-D weight pools closed here
            actq = rstack.enter_context(
                tc.tile_pool(name="actq", bufs=1, side="right"))
            qTn = actq.tile([P, HG, T], bf16, tag="qTn")
            qTr = actq.tile([P, HG // 2, T], bf16, tag="qTr")

            # ---- stage B: qT per head. Both operands fp8 -> DoubleRow
            # matmuls at 0.5 cycles/row; the weights are split hi + lo/256
            # (both fp8) for full bf16-level accuracy, combined out of two
            # PSUM accumulators by one DVE scalar_tensor_tensor. ----
            with (
                tc.tile_pool(name="wqbp", bufs=4) as wqbp,
                tc.tile_pool(name="qswp", bufs=1) as qswp,
                tc.tile_pool(name="lobp", bufs=3) as lobp,
            ):

                def emit_bunit(unit, wdram, widx, dst, dslice):
                    if unit == 0:
                        whi, wlo = wqb0
                    else:
                        whi = wqbp.tile(
                            [P, QCH, P], f8, tag="wqb", name="wqb_hi")
                        nc.sync.dma_start(whi[:], wdram[0, widx, :, :, :])
                        wlo = wqbp.tile(
                            [P, QCH, P], f8, tag="wqb", name="wqb_lo")
                        nc.sync.dma_start(wlo[:], wdram[1, widx, :, :, :])
                    for t in range(NT):
                        # rotate the hi accumulator across the mm and the
                        # (idle during B) pd pools: the ACT+DVE combine chain
                        # is ~as long as one unit's matmuls, so 3-deep
                        # rotation alone stalls the PE
                        php = psum if (2 * unit + t) % 2 == 0 else pspd
                        ph = php.tile(
                            [P, TQ], f32,
                            tag="mm" if php is psum else "pd", name="psBh")
                        pl = pssc.tile([P, TQ], f32, tag="psc", name="psBl")
                        for k2 in range(QCH // 2):
                            rhs = qlat_all[
                                :, 2 * k2 : 2 * k2 + 2, t * TQ : (t + 1) * TQ]
                            nc.tensor.matmul(
                                ph[:],
                                whi[:, 2 * k2 : 2 * k2 + 2, :],
                                rhs,
                                start=(k2 == 0),
                                stop=(k2 == QCH // 2 - 1),
                                perf_mode=DR,
                            )
                            nc.tensor.matmul(
                                pl[:],
                                wlo[:, 2 * k2 : 2 * k2 + 2, :],
                                rhs,
                                start=(k2 == 0),
                                stop=(k2 == QCH // 2 - 1),
                                perf_mode=DR,
                            )
                        # two PSUM reads in one DVE op fail the walrus
                        # verifier -> ACT scales lo to SBUF, DVE adds
                        lo_sb = lobp.tile(
                            [P, TQ], bf16, tag="losb", name="lo_sb")
                        nc.scalar.mul(lo_sb[:], pl[:], 1.0 / 256.0)
                        nc.vector.tensor_tensor(
                            dst[:, dslice, t * TQ : (t + 1) * TQ],
                            lo_sb[:], ph[:], ALU.add)

                for h in range(HG):
                    emit_bunit(h, wqbn, h, qTn, h)
                for p_ in range(HG // 2):
                    emit_bunit(HG + p_, wqbr, p_, qTr, p_)
                    # rope this pair immediately (overlaps next pair)
                    qsw = qswp.tile([P, T], bf16, tag="qsw", name="qsw")
                    qp = qTr[:, p_, :]
                    nc.sync.dma_start(qsw[0:32, :], qp[32:64, :])
                    nc.sync.dma_start(qsw[32:64, :], qp[0:32, :])
                    nc.sync.dma_start(qsw[64:96, :], qp[96:128, :])
                    nc.sync.dma_start(qsw[96:128, :], qp[64:96, :])
                    nc.vector.tensor_tensor(qsw[:], qsw[:], sin_sb[:], ALU.mult)
                    nc.vector.tensor_tensor(qp, qp, cos_sb[:], ALU.mult)
                    nc.vector.tensor_tensor(qp, qp, qsw[:], ALU.add)

            STAGE_MARKS.append(("EF", nc.next_id()))
            # ---- stages E+F per token tile (t=1 first: its leading tk
            # chunks need no causal mask, hiding the mask DMA) ----
            with (
                tc.tile_pool(name="cmp", bufs=1) as cmp_,
                tc.tile_pool(name="attp", bufs=2) as attp,
                tc.tile_pool(name="wrkE", bufs=3) as wrkE,
                tc.tile_pool(name="recp", bufs=2) as recp,
                tc.tile_pool(name="wop", bufs=8) as wop,
                tc.tile_pool(name="outs", bufs=6) as outp,
            ):
                # sliding causal mask: cm[dk, u] = 0 iff dk <= u - 384;
                # chunk variant rv uses columns [(3-rv)*128, (3-rv)*128+512)
                cm_sb = cmp_.tile([P, 7 * P], f32, tag="cm")
                nc.sync.dma_start(cm_sb[:], cmask[:, :])
                atts = {}
                for t in (1, 0):
                    attB = attp.tile(
                        [P, HG, TQ], bf16, tag="attB", name="attB")
                    attH = attp.tile(
                        [P, HG, TQ], f8, tag="attH", name="attH")
                    attL = attp.tile(
                        [P, HG, TQ], f8, tag="attL", name="attL")
                    atts[t] = (attH, attL)
                    nchunks = 4 * (t + 1)
                    # software-pipelined chunk stream across heads: the
                    # pd/av matmuls of unit i-2 run after the scores of
                    # unit i, so the PE never waits on the exp (ACT) chain
                    exs = {}
                    pdavs = {}

                    def trim(tkc):
                        # diagonal chunk rv: query columns < rv*128 are fully
                        # causal-masked -> skip them in scores/exp/pd/av; only
                        # the 128-wide triangular band needs the mask add
                        # (always cmask columns [384:512])
                        return max(0, tkc - 4 * t) * P

                    def emit_scores(h, tkc):
                        hb = 64 * (h % 2)
                        co = trim(tkc)
                        ps_ = pssc.tile([P, TQ], f32, tag="psc", name="psc")
                        nc.tensor.matmul(
                            ps_[:, co:],
                            kTn[:, h, tkc * P : (tkc + 1) * P],
                            qTn[:, h, t * TQ + co : (t + 1) * TQ],
                            start=True,
                            stop=False,
                        )
                        nc.tensor.matmul(
                            ps_[:, co:],
                            kpe2[hb : hb + 64, tkc * P : (tkc + 1) * P],
                            qTr[hb : hb + 64, h // 2,
                                t * TQ + co : (t + 1) * TQ],
                            start=False,
                            stop=True,
                        )
                        if tkc >= 4 * t:
                            nc.vector.tensor_tensor(
                                ps_[:, co : co + P],
                                ps_[:, co : co + P],
                                cm_sb[:, 3 * P : 4 * P],
                                ALU.add,
                            )
                        ex = wrkE.tile([P, TQ], bf16, tag="exp", name="ex")
                        nc.scalar.activation(
                            ex[:, co:],
                            ps_[:, co:],
                            AF.Exp,
                            bias=kb_sb[:, tkc : tkc + 1],
                            scale=SCALE,
                        )
                        exs[(h, tkc)] = ex

                    def emit_pdav(h, tkc):
                        if tkc == 0:
                            pdavs[h] = (
                                pspd.tile([P, TQ], f32, tag="pd", name="pd"),
                                psum.tile([P, TQ], f32, tag="mm", name="pav"),
                            )
                        pd, pav = pdavs[h]
                        co = trim(tkc)
                        ex = exs.pop((h, tkc))
                        nc.tensor.matmul(
                            pd[:, co:],
                            ones_sb[:],
                            ex[:, co:],
                            start=(tkc == 0),
                            stop=(tkc == nchunks - 1),
                            skip_group_check=True,
                        )
                        nc.tensor.matmul(
                            pav[:, co:],
                            vq[h // 4][:, tkc, (h % 4) * P : (h % 4 + 1) * P],
                            ex[:, co:],
                            start=(tkc == 0),
                            stop=(tkc == nchunks - 1),
                            skip_group_check=True,
                        )
                        if tkc == nchunks - 1:
                            rec = recp.tile([P, TQ], f32, tag="rec", name="rec")
                            with nc.allow_low_precision("softmax denom"):
                                nc.vector.reciprocal(rec[:], pd[:])
                            nc.vector.tensor_tensor(
                                attB[:, h, :], pav[:], rec[:], ALU.mult)
                            # hi/lo fp8 split of the attention output for the
                            # DoubleRow output projection; on the otherwise
                            # idle gpsimd engine (SBUF-only operands)
                            dsc = recp.tile([P, TQ], bf16, tag="dsc", name="dsc")
                            nc.gpsimd.tensor_copy(attH[:, h, :], attB[:, h, :])
                            nc.gpsimd.tensor_tensor(
                                dsc[:], attB[:, h, :], attH[:, h, :],
                                ALU.subtract)
                            nc.gpsimd.tensor_scalar_mul(
                                attL[:, h, :], dsc[:], 256.0)

                    LAG = 3
                    units = [
                        (h, tkc) for h in range(HG) for tkc in range(nchunks)]
                    for i, (h, tkc) in enumerate(units):
                        emit_scores(h, tkc)
                        if i >= LAG:
                            emit_pdav(*units[i - LAG])
                    for i in range(len(units) - LAG, len(units)):
                        emit_pdav(*units[i])

                # ---- output projection, hi/lo fp8 DoubleRow: po_main gets
                # attH x woH; po_mid gets attH x woL + attL x woH (both
                # 256x-scaled); lo*lo is negligible and dropped. nt-outer /
                # t-inner so each wo tile is loaded once, not once per t ----
                for nt in range(NHID):
                    whs, wls = [], []
                    for half in range(2):
                        wh_ = wop.tile([P, 4, TQ], f8, tag="wo", name="wh_")
                        nc.sync.dma_start(wh_[:], wo[0, nt, half, :, :, :])
                        wl_ = wop.tile([P, 4, TQ], f8, tag="wo", name="wl_")
                        nc.sync.dma_start(wl_[:], wo[1, nt, half, :, :, :])
                        whs.append(wh_)
                        wls.append(wl_)
                    for t in (1, 0):
                        attH, attL = atts[t]
                        for tqc in range(TQ // P):
                            pom = psum.tile([P, TQ], f32, tag="mm", name="pom")
                            pol = pssc.tile([P, TQ], f32, tag="psc", name="pol")
                            for j in range(HG // 2):
                                lhsH = attH[:, 2 * j : 2 * j + 2,
                                            tqc * P : (tqc + 1) * P]
                                lhsL = attL[:, 2 * j : 2 * j + 2,
                                            tqc * P : (tqc + 1) * P]
                                rhsH = whs[j // 2][
                                    :, 2 * (j % 2) : 2 * (j % 2) + 2, :]
                                rhsL = wls[j // 2][
                                    :, 2 * (j % 2) : 2 * (j % 2) + 2, :]
                                nc.tensor.matmul(
                                    pom[:], lhsH, rhsH,
                                    start=(j == 0), stop=(j == HG // 2 - 1),
                                    perf_mode=DR)
                                nc.tensor.matmul(
                                    pol[:], lhsH, rhsL,
                                    start=(j == 0), stop=False,
                                    perf_mode=DR)
                                nc.tensor.matmul(
                                    pol[:], lhsL, rhsH,
                                    start=False, stop=(j == HG // 2 - 1),
                                    perf_mode=DR)
                            mid_sb = outp.tile(
                                [P, TQ], bf16, tag="mid", name="mid_sb")
                            nc.scalar.mul(mid_sb[:], pol[:], 1.0 / 256.0)
                            ot = outp.tile(
                                [P, TQ], mybir.dt.float16, tag="osb", name="ot")
                            nc.vector.tensor_tensor(
                                ot[:], mid_sb[:], pom[:], ALU.add)
                            nc.sync.dma_start(out[t, nt, tqc, :, :], ot[:])

    nc.finalize()
    return nc


def _get_program():
    if "nc" not in _CACHED:
        _CACHED["nc"] = _build_program()
    return _CACHED["nc"]


def _host_prep(x, wq_a, q_norm_w, wq_b, wkv_a, kv_norm_w, wkv_b, wo,
               attention_mask, positions):
    """Build the 8 per-core input maps (bf16 weights, partition-major)."""
    import ml_dtypes

    f = np.float32
    bf = ml_dtypes.bfloat16

    def hilo(w):
        # fp8 e4m3 hi + (residual*256) lo pair; exact to ~1e-3 of w
        f8t = ml_dtypes.float8_e4m3
        w = np.ascontiguousarray(w).astype(f)
        hi = w.astype(f8t)
        lo = ((w - hi.astype(f)) * 256.0).astype(f8t)
        return np.ascontiguousarray(np.stack([hi, lo]))

    x = np.asarray(x, f)
    wq_a = np.asarray(wq_a, f)
    wkv_a = np.asarray(wkv_a, f)
    # norm weights folded into the up-projection rows
    wq_b3 = (np.asarray(wq_b, f) * np.asarray(q_norm_w, f)[:, None]) \
        .reshape(Q_RANK, H, D_QK)
    wkv_b3 = (np.asarray(wkv_b, f) * np.asarray(kv_norm_w, f)[:, None]) \
        .reshape(KV_RANK, H, D_NOPE + D_V)
    wo2 = np.asarray(wo, f)
    attention_mask = np.asarray(attention_mask)
    positions = np.asarray(positions)

    # A-q weights chunk-major, hi/lo packed per chunk:
    # wqac[m, p, hl, kk, col] = hilo(wq_a)[hl, kk*128+p, m*128+col]
    wqac = np.ascontiguousarray(
        hilo(wq_a.reshape(HIDK, P, QCH, P).transpose(2, 1, 0, 3))
        .transpose(1, 2, 0, 3, 4))

    # A-kv weights quart-major (as original)
    wkva_blk = hilo(
        wkv_a[:, :KV_RANK].reshape(4, KQ, P, 2, 256).transpose(3, 0, 2, 1, 4))
    wkvr_blk = hilo(
        wkv_a[:, KV_RANK:].reshape(4, KQ, P, 64).transpose(0, 2, 1, 3))

    inv_freq = 1.0 / (
        THETA ** (np.arange(0, D_ROPE, 2, dtype=np.float64) / D_ROPE))

    dk = np.arange(P)[:, None]
    u = np.arange(7 * P)[None, :]
    cmask_ = np.where(dk <= u - 3 * P, 0.0, NMASK).astype(f)
    onesd = np.ones((P, P), bf)

    per_batch = {}
    for b in range(B):
        # xT[hl, k2, p, j, t] = hilo(x.T)[hl, (2*k2+j)*128+p, t]
        xTb = np.ascontiguousarray(
            hilo(x[b].T.reshape(HIDK // 2, 2, P, T).transpose(0, 2, 1, 3)))
        ang = positions[b].astype(np.float64)[:, None] * inv_freq[None, :]
        cosT = np.cos(ang).astype(f).T  # [32, T]
        sinT = np.sin(ang).astype(f).T
        cos4_ = np.ascontiguousarray(np.tile(cosT, (4, 1))).astype(bf)
        sin4_ = np.ascontiguousarray(
            np.concatenate([-sinT, sinT, -sinT, sinT], axis=0)).astype(bf)
        kb = np.where(attention_mask[b] != 0, 0.0, NMASK).astype(f)
        kbias_ = np.ascontiguousarray(kb.reshape(TC, P).T)
        per_batch[b] = (xTb, cos4_, sin4_, kbias_)

    in_maps = []
    for c in range(8):
        b, g = c // 4, c % 4
        hs = slice(g * HG, (g + 1) * HG)
        xTb, cos4_, sin4_, kbias_ = per_batch[b]
        # local x slab: xloc[hl, p, kk, col] = xT[hl, kk*128+p, g*TL+col]
        xloc_ = hilo(
            x[b][g * TL : (g + 1) * TL, :].T
            .reshape(HIDK, P, TL).transpose(1, 0, 2))

        wqbn_ = hilo(
            wq_b3[:, hs, :D_NOPE]
            .reshape(QCH, P, HG, P).transpose(2, 1, 0, 3))
        # rope cols packed in head pairs: [h_even 64 | h_odd 64] per 128-col
        wqbr_ = hilo(
            wq_b3[:, hs, D_NOPE:]
            .reshape(QCH, P, HG // 2, P).transpose(2, 1, 0, 3))
        wkbn_ = np.ascontiguousarray(
            wkv_b3[:, hs, :D_NOPE]
            .reshape(KCH, P, HG, P).transpose(1, 2, 0, 3)).astype(bf)
        wkbv_ = np.ascontiguousarray(
            wkv_b3[:, hs, D_NOPE:]
            .reshape(KCH, P, 2, TQ).transpose(2, 1, 0, 3)).astype(bf)
        wosh = hilo(
            wo2[g * HG * D_V : (g + 1) * HG * D_V, :]
            .reshape(2, 4, P, NHID, TQ).transpose(3, 0, 2, 1, 4))
        in_maps.append({
            "xloc": xloc_, "xT": xTb, "wqac": wqac,
            "wkva": wkva_blk, "wkvr": wkvr_blk,
            "wqbn": wqbn_, "wqbr": wqbr_, "wkbn": wkbn_, "wkbv": wkbv_,
            "wo": wosh,
            "cos4": cos4_, "sin4": sin4_,
            "cmask": cmask_, "kbias": kbias_, "onesd": onesd,
        })
    return in_maps


def kernel(**inputs):
    from concourse.bass_utils import run_bass_kernel_spmd

    nc = _get_program()
    in_maps = _host_prep(**inputs)
    res = run_bass_kernel_spmd(nc, in_maps, core_ids=list(range(8)))
    _CACHED["last_result"] = res
    out = np.zeros((B, T, HID), np.float32)
    for c in range(8):
        blk = np.asarray(res.results[c]["out"], np.float32)
        # row = t*512 + q*128 + p, col = nt*512 + c
        out[c // 4] += blk.transpose(0, 2, 3, 1, 4).reshape(T, HID)
    return out


# revision 84
# speedup vs baseline: 1.0072x; 1.0072x over previous
"""DeepseekV3 MLA forward on 8 TRN2 NeuronCores.

Sharding: data-parallel over batch (B=2 -> 2 groups of 4 cores), tensor-
parallel over heads within each batch group (32 heads -> 4 groups of 8).

vs the original replicated-latents kernel (592us -> 339us):
  * RMSNorm weights folded into wq_b / wkv_b rows on the host; the device
    norm is x * rsqrt(mean(x^2)+eps), and the kv-path rstd scaling commutes
    into stage D's output copies so D starts on raw latents.
  * The q-latent projection (45% of the old FLOPs) is token-split across
    the 4 cores of each batch group: each core projects+norms its 256-token
    quarter, then a 4-core HBM AllGather (fp8, 0.39MB in / 1.6MB out per
    core) distributes the normalized q-latents. The kv path stays fully
    replicated: its compute hides the collective completely.
  * Every weight*activation GEMM runs as fp8-e4m3 DoubleRow chains with
    exact hi + lo/256 weight splitting (more accurate than bf16 weights,
    0.5 cycles/row): stages A-q/A-kv/F split both operands (3 chains, 75%
    of bf16 cost), stage B rides the already-fp8 gathered latents (2
    chains, 50%). Attention scores/av stay bf16; PSUM is always f32.
  * Causal trim: diagonal score chunks skip fully-masked query columns in
    scores/exp/denominator/av; the remaining triangular band mask is a
    single 128-col cmask window.
  * Software-pipelined attention (scores of unit i+3 ahead of pd/av of
    unit i), E(1),E(0),F(1),F(0) phase order with the attention fp8
    conversions hidden under F(1), nt-outer F loop so wo loads once,
    f16 output tensor (host upcasts).
  * Measured on HW: rel err 1.38e-2 (gate 2e-2), 336976 ns per core.

Dataflow on device keeps activations transposed ([feature, token]) so
every matmul contracts over the partition dim with no on-device
transposes anywhere:
  qlatT   = wq_a.T @ xT  (local 256 tokens, chunk-major over rank)
  kvlatT  = wkv_a.T @ xT (all tokens, quart-major over HID)
  qT_h    = wq_b_h.T @ qlatT                                   [d, T]
  scoresT = kT_h-chunks @ qT_h                                 [tk, tq]
  softmax over tk (=partitions): exp on ACT, denominator via a
  ones[128,128] matmul (result replicated across partitions), then
  attn_outT = v_chunks.T @ expT                                [dv, tq]
  out      = attnT-chunks.T @ wo_h  (natural layout)           [tq, hid]
RoPE in transposed layout: rot(x) = x*cos + swap32(x)*(+-sin), where
swap32 exchanges the two 32-row halves of each 64-row rope block (done
with SBUF->SBUF block DMAs) and the +-sin sign pattern is host-built.
"""

import os
import sys

import numpy as np

sys.path.insert(0, "/opt/trn_rl_repo")

B, T, HID = 2, 1024, 4096
H, D_NOPE, D_ROPE, D_V = 32, 128, 64, 128
D_QK = D_NOPE + D_ROPE
Q_RANK, KV_RANK = 1536, 512
THETA, EPS = 10000.0, 1e-6
SCALE = float(D_QK) ** -0.5
NMASK = -30000.0

HG = H // 4          # heads per core = 8
P = 128
QCH = Q_RANK // P    # 12 latent chunks (q)
KCH = KV_RANK // P   # 4 latent chunks (kv)
HIDK = HID // P      # 32 contraction tiles
KQ = HIDK // 4       # 8 k-tiles per quart
TQ = 512             # token tile (free dim) for most matmuls
NT = T // TQ         # 2 token tiles
TC = T // P          # 8 token chunks of 128
NHID = HID // TQ     # 8 output column tiles
TL = T // 4          # 256 local tokens per core (q path token split)

_CACHED = {}
STAGE_MARKS = []


def _build_program():
    import contextlib

    import concourse.bacc as bacc
    import concourse.mybir as mybir
    import concourse.tile as tile

    f32 = mybir.dt.float32
    bf16 = mybir.dt.bfloat16
    AF = mybir.ActivationFunctionType
    ALU = mybir.AluOpType
    DR = mybir.MatmulPerfMode.DoubleRow

    nc = bacc.Bacc()
    del STAGE_MARKS[:]

    # ---- DRAM I/O (per-core; SPMD across the 8 cores) ----
    # leading dim 2 = fp8 hi / (residual*256) lo pair
    f8_ = mybir.dt.float8e4
    xloc = nc.dram_tensor("xloc", (2, P, HIDK, TL), f8_, kind="ExternalInput")
    xT = nc.dram_tensor(
        "xT", (2, HIDK // 2, P, 2, T), f8_, kind="ExternalInput")
    wqac = nc.dram_tensor(
        "wqac", (QCH, P, 2, HIDK, P), f8_, kind="ExternalInput")
    wkva = nc.dram_tensor(
        "wkva", (2, 2, 4, P, KQ, 256), f8_, kind="ExternalInput")
    wkvr = nc.dram_tensor("wkvr", (2, 4, P, KQ, 64), f8_, kind="ExternalInput")
    f8i = mybir.dt.float8e4
    wqbn = nc.dram_tensor("wqbn", (2, HG, P, QCH, P), f8i, kind="ExternalInput")
    wqbr = nc.dram_tensor(
        "wqbr", (2, HG // 2, P, QCH, P), f8i, kind="ExternalInput")
    wkbn = nc.dram_tensor("wkbn", (P, HG, KCH, P), bf16, kind="ExternalInput")
    wkbv = nc.dram_tensor("wkbv", (2, P, KCH, TQ), bf16, kind="ExternalInput")
    wo = nc.dram_tensor(
        "wo", (2, NHID, 2, P, 4, TQ), mybir.dt.float8e4, kind="ExternalInput")
    cos4 = nc.dram_tensor("cos4", (P, T), bf16, kind="ExternalInput")
    sin4 = nc.dram_tensor("sin4", (P, T), bf16, kind="ExternalInput")  # +-sin
    cmask = nc.dram_tensor("cmask", (P, 7 * P), f32, kind="ExternalInput")
    kbias = nc.dram_tensor("kbias", (P, TC), f32, kind="ExternalInput")
    onesd = nc.dram_tensor("onesd", (P, P), bf16, kind="ExternalInput")
    f8 = mybir.dt.float8e4
    gin = nc.dram_tensor("gin", (P, QCH, TL), f8, kind="Internal")
    gout = nc.dram_tensor("gout", (4, P, QCH, TL), f8, kind="Internal")
    f16 = mybir.dt.float16
    out = nc.dram_tensor("out", (NT, NHID, 4, P, TQ), f16, kind="ExternalOutput")

    with tile.TileContext(nc) as tc, contextlib.ExitStack() as rstack:
        with (
            tc.tile_pool(name="const", bufs=1) as const,
            tc.tile_pool(name="psmm", bufs=3, space="PSUM") as psum,
            tc.tile_pool(name="pspd", bufs=2, space="PSUM") as pspd,
            tc.tile_pool(name="pssc", bufs=3, space="PSUM") as pssc,
        ):
            ones_sb = const.tile([P, P], bf16, tag="ones")
            cos_sb = const.tile([P, T], bf16, tag="cos")
            sin_sb = const.tile([P, T], bf16, tag="sin")
            kb_sb = const.tile([P, TC], f32, tag="kb")
            zero_b = const.tile([P, 1], f32, tag="zb")
            nc.vector.memset(zero_b[:], 0.0)
            eps_b = const.tile([P, 1], f32, tag="eb")
            nc.vector.memset(eps_b[:], EPS)

            def emit_const_dmas():
                # ones feeds the A-q sum-of-squares matmuls (~12us in);
                # cos/sin/kb aren't read until the kv rope / EF exp, so they
                # ride behind the quart-1 x stream instead of delaying the
                # compute-gating wqac pieces at the head of the DMA queue
                nc.sync.dma_start(ones_sb[:], onesd[:, :])

            def emit_late_const_dmas():
                nc.sync.dma_start(cos_sb[:], cos4[:, :])
                nc.sync.dma_start(sin_sb[:], sin4[:, :])
                nc.sync.dma_start(kb_sb[:], kbias[:, :])

            # long-lived activations (right side of SBUF)
            qlatp = rstack.enter_context(
                tc.tile_pool(name="qlatp", bufs=1, side="right"))
            qlat_all = qlatp.tile([P, QCH, T], f8, tag="qlat")
            kpep = rstack.enter_context(
                tc.tile_pool(name="kpep", bufs=1, side="right"))
            kpe2 = kpep.tile([P, T], bf16, tag="kpe2")
            kTnp = rstack.enter_context(
                tc.tile_pool(name="kTnp", bufs=1, side="right"))
            kTn = kTnp.tile([P, HG, T], bf16, tag="kTn")
            vqp = rstack.enter_context(
                tc.tile_pool(name="vqp", bufs=2, side="right"))
            vq = [
                vqp.tile([P, TC, 4 * D_V], bf16, tag="vq", name="vq")
                for _ in range(2)
            ]

            wqb0p = rstack.enter_context(
                tc.tile_pool(name="wqb0p", bufs=1, side="right"))
            wqb0 = (
                wqb0p.tile([P, QCH, P], f8, tag="wqb0h", name="wqb0h"),
                wqb0p.tile([P, QCH, P], f8, tag="wqb0l", name="wqb0l"),
            )

            qtmpp = rstack.enter_context(
                tc.tile_pool(name="qtmpp", bufs=1, side="right"))

            # kv-path x tiles: pool opened early so quart 0/1 loads can be
            # issued while stage A-q still computes (SP queue is in-order);
            # closed right after stage D to free SBUF for the EF stage
            xk_stack = contextlib.ExitStack()
            xkp = xk_stack.enter_context(
                tc.tile_pool(name="xk", bufs=12, side="right"))
            wAp = xk_stack.enter_context(
                tc.tile_pool(name="wA", bufs=2, side="right"))

            def load_xk(quart):
                # paired tiles [P, 2, T] so DoubleRow sees adjacent k-tiles
                tiles = []
                for k2 in range(KQ // 2):
                    kp = quart * (KQ // 2) + k2
                    xh_ = xkp.tile([P, 2, T], f8, tag="xkh", name="xh_")
                    nc.sync.dma_start(xh_[:], xT[0, kp, :, :, :])
                    xl_ = xkp.tile([P, 2, T], f8, tag="xkl", name="xl_")
                    nc.sync.dma_start(xl_[:], xT[1, kp, :, :, :])
                    tiles.append((xh_, xl_))
                return tiles

            kvblocks = [
                (wkva, 0, 256, 0),
                (wkva, 1, 256, 2),
                (wkvr, None, 64, 4),
            ]

            def emit_wA(quart):
                # tiles sized to the block width so every DMA lands in
                # contiguous >=512B runs (a [:, :, :64] slice of a 256-wide
                # tile would write 64B strided runs at 2x latency)
                wts = []
                for bi, (wdram, blki, width, dch) in enumerate(kvblocks):
                    pair = []
                    for hl_ in range(2):
                        wt = wAp.tile(
                            [P, KQ, width], f8, tag=f"wA{bi}{hl_}", name="wt")
                        wsrc = (
                            wdram[hl_, quart, :, :, :]
                            if blki is None
                            else wdram[hl_, blki, quart, :, :, :]
                        )
                        nc.sync.dma_start(wt[:], wsrc)
                        pair.append(wt)
                    wts.append(pair)
                return wts

            # ---- stage A-q: local-quarter q latents, chunk-major ----
            with (
                tc.tile_pool(name="xlocp", bufs=1) as xlocp,
                tc.tile_pool(name="wqap", bufs=3) as wqap,

                tc.tile_pool(name="wrkq", bufs=3) as wrkq,
            ):
                # PE p-state warm-up: dummy matmuls on a memset tile while
                # the first real DMAs land, so real matmuls start at full
                # clock instead of paying the 3us ramp
                warm = wrkq.tile([P, TQ], bf16, tag="warm", name="warm")
                nc.vector.memset(warm[:], 0.0)
                wps = pssc.tile([P, TQ], f32, tag="psc", name="wps")
                for _ in range(26):
                    nc.tensor.matmul(
                        wps[:], warm[:, :P], warm[:], start=True, stop=True)

                xh_sb = xlocp.tile([P, HIDK, TL], f8, tag="xlh")
                nc.sync.dma_start(xh_sb[:], xloc[0, :, :, :])
                xl_sb = xlocp.tile([P, HIDK, TL], f8, tag="xll")
                qtmp = qtmpp.tile([P, QCH, TL], bf16, tag="qtmp")
                # fp8 staging for the gather: quantized once, from the
                # normalized bf16 values (scores-only path; validated at
                # 1.4e-2 max-rel-err vs the 2e-2 gate)
                qgin = qtmpp.tile([P, QCH, TL], f8, tag="qgin")
                ssq = pspd.tile([P, TQ], f32, tag="pd", name="ssq")
                prev_sq = None
                for m in range(QCH):
                    whl = wqap.tile([P, 2, HIDK, P], f8, tag="wqa", name="whl")
                    nc.sync.dma_start(whl[:], wqac[m, :, :, :, :])
                    if m == 0:
                        nc.sync.dma_start(xl_sb[:], xloc[1, :, :, :])
                        emit_const_dmas()
                    if m == 8:
                        # stage A-kv quart-0 x tiles prefetched behind the
                        # remaining (compute-gated) wqac loads, so the kv
                        # path starts the moment A-q's matmuls finish
                        xk_pre = [load_xk(0)]
                    ps = psum.tile([P, TQ], f32, tag="mm", name="psq")
                    psl = pssc.tile([P, TQ], f32, tag="psc", name="psql")
                    # main chain first: the mid chains' xl operand lands
                    # a few us after xh, so interleaving stalls chunk 0
                    for k2 in range(HIDK // 2):
                        sl2 = slice(2 * k2, 2 * k2 + 2)
                        nc.tensor.matmul(
                            ps[:, :TL], whl[:, 0, sl2, :], xh_sb[:, sl2, :],
                            start=(k2 == 0), stop=(k2 == HIDK // 2 - 1),
                            perf_mode=DR)
                    for k2 in range(HIDK // 2):
                        sl2 = slice(2 * k2, 2 * k2 + 2)
                        nc.tensor.matmul(
                            psl[:, :TL], whl[:, 0, sl2, :], xl_sb[:, sl2, :],
                            start=(k2 == 0), stop=False, perf_mode=DR)
                        nc.tensor.matmul(
                            psl[:, :TL], whl[:, 1, sl2, :], xh_sb[:, sl2, :],
                            start=False, stop=(k2 == HIDK // 2 - 1),
                            perf_mode=DR)
                    if prev_sq is not None:
                        nc.tensor.matmul(
                            ssq[:, :TL],
                            ones_sb[:],
                            prev_sq[:],
                            start=(m == 1),
                            stop=False,
                        )
                    mid_sb = wrkq.tile([P, TL], bf16, tag="mid", name="mid_sb")
                    nc.scalar.mul(mid_sb[:], psl[:, :TL], 1.0 / 256.0)
                    nc.vector.tensor_tensor(
                        qtmp[:, m, :], mid_sb[:], ps[:, :TL], ALU.add)
                    sq = wrkq.tile([P, TL], bf16, tag="sq", name="sq")
                    nc.scalar.activation(
                        sq[:], qtmp[:, m, :], AF.Square, bias=zero_b[:])
                    prev_sq = sq
                nc.tensor.matmul(
                    ssq[:, :TL], ones_sb[:], prev_sq[:], start=False, stop=True)
                # quart-0 kv weights + second x quart: issued right after
                # the last compute-gated wqac load
                wA_pre = emit_wA(0)
                xk_pre.append(load_xk(1))
                emit_late_const_dmas()
                std = wrkq.tile([P, TL], f32, tag="std", name="std")
                nc.scalar.activation(
                    std[:], ssq[:, :TL], AF.Sqrt, bias=eps_b[:],
                    scale=1.0 / Q_RANK)
                rstd = qtmpp.tile([P, TL], f32, tag="rstd", name="rstd")
                with nc.allow_low_precision("rmsnorm rstd"):
                    nc.vector.reciprocal(rstd[:], std[:])

                def emit_qnorm_gather():
                    # emitted after A-kv quart 1: on the DVE queue these
                    # mults wait on rstd, and emitted any earlier they stall
                    # the quart combines (and the PSUM rotation) behind them
                    for m in range(QCH):
                        nc.vector.tensor_tensor(
                            qgin[:, m, :], qtmp[:, m, :], rstd[:], ALU.mult)
                    nc.sync.dma_start(gin[:, :, :], qgin[:, :, :])
                    nc.gpsimd.collective_compute(
                        "AllGather",
                        mybir.AluOpType.bypass,
                        replica_groups=[[0, 1, 2, 3], [4, 5, 6, 7]],
                        ins=[gin[:, :, :]],
                        outs=[gout[:, :, :, :]],
                    )

            STAGE_MARKS.append(("Akv", nc.next_id()))
            # ---- stage A-kv: full-token kv latents, quart-major ----
            with (
                tc.tile_pool(name="kvlatp", bufs=1) as kvlatp,
                tc.tile_pool(name="kvnp", bufs=1) as kvnp,
                tc.tile_pool(name="rstdp", bufs=2) as rstdp,
                tc.tile_pool(name="wkbnp", bufs=1) as wkbnp,
                tc.tile_pool(name="wkbvp", bufs=1) as wkbvp,
                tc.tile_pool(name="wrkA", bufs=3) as wrkA,
            ):
                kvlat = [
                    kvlatp.tile([P, T], bf16, tag=f"kvlat{i}", name=f"kvlat{i}")
                    for i in range(KCH + 1)
                ]
                kvn = [
                    kvnp.tile([P, T], bf16, tag=f"kvn{i}", name=f"kvn{i}")
                    for i in range(KCH)
                ]
                wkn = wkbnp.tile([P, HG, KCH, P], bf16, tag="wkbn", name="wkbn")
                wkvts = [
                    wkbvp.tile([P, KCH, TQ], bf16, tag=f"wkbv{q_}", name="wkbv")
                    for q_ in range(2)
                ]
                def emit_dweight_dmas():
                    nc.sync.dma_start(wkn[:], wkbn[:, :, :, :])
                    for q_ in range(2):
                        nc.sync.dma_start(wkvts[q_][:], wkbv[q_, :, :, :])
                    nc.sync.dma_start(wqb0[0][:], wqbn[0, 0, :, :, :])
                    nc.sync.dma_start(wqb0[1][:], wqbn[1, 0, :, :, :])

                if True:
                    for quart in range(4):
                        xk = xk_pre[quart] if quart < 2 else load_xk(quart)
                        wts_q = wA_pre if quart == 0 else emit_wA(quart)
                        for mbi, (wdram, blki, width, dch) in \
                                enumerate(kvblocks):
                            nm = (width + P - 1) // P
                            wth, wtl = wts_q[mbi]
                            for m in range(nm):
                                w = min(width - m * P, P)
                                msl = slice(m * P, m * P + w)
                                for t in range(NT):
                                    tsl = slice(t * TQ, (t + 1) * TQ)
                                    psm = psum.tile(
                                        [P, TQ], f32, tag="mm", name="psAm")
                                    psl = pssc.tile(
                                        [P, TQ], f32, tag="psc", name="psAl")
                                    for k2 in range(KQ // 2):
                                        s2 = slice(2 * k2, 2 * k2 + 2)
                                        xh2, xl2 = xk[k2]
                                        nc.tensor.matmul(
                                            psm[:w, :],
                                            wth[:, s2, msl],
                                            xh2[:, :, tsl],
                                            start=(k2 == 0),
                                            stop=(k2 == KQ // 2 - 1),
                                            perf_mode=DR,
                                        )
                                        nc.tensor.matmul(
                                            psl[:w, :],
                                            wth[:, s2, msl],
                                            xl2[:, :, tsl],
                                            start=(k2 == 0),
                                            stop=False,
                                            perf_mode=DR,
                                        )
                                        nc.tensor.matmul(
                                            psl[:w, :],
                                            wtl[:, s2, msl],
                                            xh2[:, :, tsl],
                                            start=False,
                                            stop=(k2 == KQ // 2 - 1),
                                            perf_mode=DR,
                                        )
                                    dst = kvlat[dch + m][:w, tsl]
                                    mids = wrkA.tile(
                                        [P, TQ], bf16, tag="mids", name="mids")
                                    nc.scalar.mul(
                                        mids[:w, :], psl[:w, :], 1.0 / 256.0)
                                    if quart == 0:
                                        nc.vector.tensor_tensor(
                                            dst, mids[:w, :], psm[:w, :],
                                            ALU.add)
                                    else:
                                        nc.gpsimd.tensor_tensor(
                                            dst, dst, mids[:w, :], ALU.add)
                                        nc.vector.tensor_tensor(
                                            dst, psm[:w, :], dst, ALU.add)
                        if quart == 1:
                            emit_qnorm_gather()


                    emit_dweight_dmas()
                    STAGE_MARKS.append(("Anorm", nc.next_id()))
                    # ---- kv rmsnorm (w folded on host): rstd only; the
                    # normalize is folded into stage D's output copies (kT)
                    # and into separate kvn tiles (v path), so D's matmuls
                    # start immediately on the raw latents ----
                    rstdvs = []
                    for t in range(NT):
                        ssp = pspd.tile([P, TQ], f32, tag="pd", name="ssp")
                        for m in range(KCH):
                            sq = wrkA.tile([P, TQ], bf16, tag="sqv", name="sqv")
                            nc.scalar.activation(
                                sq[:],
                                kvlat[m][:, t * TQ : (t + 1) * TQ],
                                AF.Square,
                                bias=zero_b[:],
                            )
                            nc.tensor.matmul(
                                ssp[:],
                                ones_sb[:],
                                sq[:],
                                start=(m == 0),
                                stop=(m == KCH - 1),
                            )
                        stdv = wrkA.tile([P, TQ], f32, tag="stdv", name="stdv")
                        nc.scalar.activation(
                            stdv[:], ssp[:], AF.Sqrt, bias=eps_b[:],
                            scale=1.0 / KV_RANK)
                        rstdv = rstdp.tile(
                            [P, TQ], f32, tag="rstdv", name="rstdv")
                        with nc.allow_low_precision("rmsnorm rstd"):
                            nc.vector.reciprocal(rstdv[:], stdv[:])
                        rstdvs.append(rstdv)
                    ksw = xkp.tile([P, T], bf16, tag="xk", name="ksw")

                STAGE_MARKS.append(("D", nc.next_id()))

                # ---- stage D: kT_nope per head (raw latents, rstd-scaled
                # on the way out), v per quad (normalized kvn copies) ----
                def emit_dkt(h):
                    pp, ptag = (psum, "mm") if h % 2 == 0 else (pssc, "psc")
                    pst = [
                        pp.tile([P, TQ], f32, tag=ptag, name="psD")
                        for _ in range(NT)
                    ]
                    for k in range(KCH):
                        for t in range(NT):
                            nc.tensor.matmul(
                                pst[t][:],
                                wkn[:, h, k, :],
                                kvlat[k][:, t * TQ : (t + 1) * TQ],
                                start=(k == 0),
                                stop=(k == KCH - 1),
                            )
                    for t in range(NT):
                        nc.vector.tensor_tensor(
                            kTn[:, h, t * TQ : (t + 1) * TQ], pst[t][:],
                            rstdvs[t][:], ALU.mult)

                for h in range(4):
                    emit_dkt(h)
                # kv normalize: DVE work emitted here so it rides behind the
                # first kT copies and finishes just before stage D's v part
                for m in range(KCH):
                    for t in range(NT):
                        nc.vector.tensor_tensor(
                            kvn[m][:, t * TQ : (t + 1) * TQ],
                            kvlat[m][:, t * TQ : (t + 1) * TQ],
                            rstdvs[t][:], ALU.mult)
                for h in range(4, HG):
                    emit_dkt(h)
                # k rope (needed first by stage EF - emit late on DVE)
                nc.sync.dma_start(kpe2[0:64, :], kvlat[KCH][0:64, :])
                nc.sync.dma_start(kpe2[64:128, :], kvlat[KCH][0:64, :])
                nc.sync.dma_start(ksw[0:32, :], kvlat[KCH][32:64, :])
                nc.sync.dma_start(ksw[32:64, :], kvlat[KCH][0:32, :])
                nc.sync.dma_start(ksw[64:96, :], kvlat[KCH][32:64, :])
                nc.sync.dma_start(ksw[96:128, :], kvlat[KCH][0:32, :])
                nc.vector.tensor_tensor(ksw[:], ksw[:], sin_sb[:], ALU.mult)
                nc.vector.tensor_tensor(
                    kpe2[:], kpe2[:], cos_sb[:], ALU.mult)
                rope_done = nc.vector.tensor_tensor(
                    kpe2[:], kpe2[:], ksw[:], ALU.add)
                for quad in range(2):
                    for tkc in range(TC):
                        pp, ptag = (
                            (psum, "mm") if tkc % 2 == 0 else (pssc, "psc"))
                        ps_ = pp.tile([P, TQ], f32, tag=ptag, name="psV")
                        for k in range(KCH):
                            nc.tensor.matmul(
                                ps_[:],
                                kvn[k][:, tkc * P : (tkc + 1) * P],
                                wkvts[quad][:, k, :],
                                start=(k == 0),
                                stop=(k == KCH - 1),
                            )
                        nc.vector.tensor_copy(vq[quad][:, tkc, :], ps_[:])

                # gathered q latents -> qlat_all (waits on the collective).
                # Explicit scheduling dep on the k-rope: without it the tile
                # scheduler hoists these ready-early-but-long-wait DMAs ahead
                # of the not-yet-ready kv weight loads on the in-order SP
                # queue, stalling stage A-kv behind the collective.
                from concourse.tile import add_dep_helper
                for g in range(4):
                    gd = nc.sync.dma_start(
                        qlat_all[:, :, g * TL : (g + 1) * TL],
                        gout[g, :, :, :],
                    )
                    add_dep_helper(
                        gd.ins, rope_done.ins, reason="gout after kv path")

            xk_stack.close()
            STAGE_MARKS.append(("B", nc.next_id()))
            # kvlat + stage-D weight pools closed here
            actq = rstack.enter_context(
                tc.tile_pool(name="actq", bufs=1, side="right"))
            qTn = actq.tile([P, HG, T], bf16, tag="qTn")
            qTr = actq.tile([P, HG // 2, T], bf16, tag="qTr")

            # ---- stage B: qT per head. Both operands fp8 -> DoubleRow
            # matmuls at 0.5 cycles/row; the weights are split hi + lo/256
            # (both fp8) for full bf16-level accuracy, combined out of two
            # PSUM accumulators by one DVE scalar_tensor_tensor. ----
            with (
                tc.tile_pool(name="wqbp", bufs=6) as wqbp,
                tc.tile_pool(name="qswp", bufs=1) as qswp,
                tc.tile_pool(name="lobp", bufs=3) as lobp,
            ):

                def emit_bunit(unit, wdram, widx, dst, dslice):
                    if unit == 0:
                        whi, wlo = wqb0
                    else:
                        whi = wqbp.tile(
                            [P, QCH, P], f8, tag="wqb", name="wqb_hi")
                        nc.sync.dma_start(whi[:], wdram[0, widx, :, :, :])
                        wlo = wqbp.tile(
                            [P, QCH, P], f8, tag="wqb", name="wqb_lo")
                        nc.sync.dma_start(wlo[:], wdram[1, widx, :, :, :])
                    for t in range(NT):
                        # rotate the hi accumulator across the mm and the
                        # (idle during B) pd pools: the ACT+DVE combine chain
                        # is ~as long as one unit's matmuls, so 3-deep
                        # rotation alone stalls the PE
                        php = psum if (2 * unit + t) % 2 == 0 else pspd
                        ph = php.tile(
                            [P, TQ], f32,
                            tag="mm" if php is psum else "pd", name="psBh")
                        pl = pssc.tile([P, TQ], f32, tag="psc", name="psBl")
                        for k2 in range(QCH // 2):
                            rhs = qlat_all[
                                :, 2 * k2 : 2 * k2 + 2, t * TQ : (t + 1) * TQ]
                            nc.tensor.matmul(
                                ph[:],
                                whi[:, 2 * k2 : 2 * k2 + 2, :],
                                rhs,
                                start=(k2 == 0),
                                stop=(k2 == QCH // 2 - 1),
                                perf_mode=DR,
                            )
                            nc.tensor.matmul(
                                pl[:],
                                wlo[:, 2 * k2 : 2 * k2 + 2, :],
                                rhs,
                                start=(k2 == 0),
                                stop=(k2 == QCH // 2 - 1),
                                perf_mode=DR,
                            )
                        # two PSUM reads in one DVE op fail the walrus
                        # verifier -> ACT scales lo to SBUF, DVE adds
                        lo_sb = lobp.tile(
                            [P, TQ], bf16, tag="losb", name="lo_sb")
                        nc.scalar.mul(lo_sb[:], pl[:], 1.0 / 256.0)
                        nc.vector.tensor_tensor(
                            dst[:, dslice, t * TQ : (t + 1) * TQ],
                            lo_sb[:], ph[:], ALU.add)

                for h in range(HG):
                    emit_bunit(h, wqbn, h, qTn, h)
                for p_ in range(HG // 2):
                    emit_bunit(HG + p_, wqbr, p_, qTr, p_)
                    # rope this pair immediately (overlaps next pair)
                    qsw = qswp.tile([P, T], bf16, tag="qsw", name="qsw")
                    qp = qTr[:, p_, :]
                    nc.sync.dma_start(qsw[0:32, :], qp[32:64, :])
                    nc.sync.dma_start(qsw[32:64, :], qp[0:32, :])
                    nc.sync.dma_start(qsw[64:96, :], qp[96:128, :])
                    nc.sync.dma_start(qsw[96:128, :], qp[64:96, :])
                    nc.vector.tensor_tensor(qsw[:], qsw[:], sin_sb[:], ALU.mult)
                    nc.vector.tensor_tensor(qp, qp, cos_sb[:], ALU.mult)
                    nc.vector.tensor_tensor(qp, qp, qsw[:], ALU.add)

            STAGE_MARKS.append(("EF", nc.next_id()))
            # ---- stages E+F per token tile (t=1 first: its leading tk
            # chunks need no causal mask, hiding the mask DMA) ----
            with (
                tc.tile_pool(name="cmp", bufs=1) as cmp_,
                tc.tile_pool(name="attp", bufs=2) as attp,
                tc.tile_pool(name="wrkE", bufs=3) as wrkE,
                tc.tile_pool(name="recp", bufs=2) as recp,
                tc.tile_pool(name="wop", bufs=8) as wop,
                tc.tile_pool(name="outs", bufs=6) as outp,
            ):
                # sliding causal mask: cm[dk, u] = 0 iff dk <= u - 384;
                # chunk variant rv uses columns [(3-rv)*128, (3-rv)*128+512)
                cm_sb = cmp_.tile([P, 7 * P], f32, tag="cm")
                nc.sync.dma_start(cm_sb[:], cmask[:, :])
                atts = {}
                for t in (1, 0):
                    attB = attp.tile(
                        [P, HG, TQ], bf16, tag="attB", name="attB")
                    attH = attp.tile(
                        [P, HG, TQ], f8, tag="attH", name="attH")
                    attL = attp.tile(
                        [P, HG, TQ], f8, tag="attL", name="attL")
                    atts[t] = (attH, attL)
                    nchunks = 4 * (t + 1)
                    # software-pipelined chunk stream across heads: the
                    # pd/av matmuls of unit i-2 run after the scores of
                    # unit i, so the PE never waits on the exp (ACT) chain
                    exs = {}
                    pdavs = {}

                    def trim(tkc):
                        # diagonal chunk rv: query columns < rv*128 are fully
                        # causal-masked -> skip them in scores/exp/pd/av; only
                        # the 128-wide triangular band needs the mask add
                        # (always cmask columns [384:512])
                        return max(0, tkc - 4 * t) * P

                    def emit_scores(h, tkc):
                        hb = 64 * (h % 2)
                        co = trim(tkc)
                        ps_ = pssc.tile([P, TQ], f32, tag="psc", name="psc")
                        nc.tensor.matmul(
                            ps_[:, co:],
                            kTn[:, h, tkc * P : (tkc + 1) * P],
                            qTn[:, h, t * TQ + co : (t + 1) * TQ],
                            start=True,
                            stop=False,
                        )
                        nc.tensor.matmul(
                            ps_[:, co:],
                            kpe2[hb : hb + 64, tkc * P : (tkc + 1) * P],
                            qTr[hb : hb + 64, h // 2,
                                t * TQ + co : (t + 1) * TQ],
                            start=False,
                            stop=True,
                        )
                        if tkc >= 4 * t:
                            nc.vector.tensor_tensor(
                                ps_[:, co : co + P],
                                ps_[:, co : co + P],
                                cm_sb[:, 3 * P : 4 * P],
                                ALU.add,
                            )
                        ex = wrkE.tile([P, TQ], bf16, tag="exp", name="ex")
                        nc.scalar.activation(
                            ex[:, co:],
                            ps_[:, co:],
                            AF.Exp,
                            bias=kb_sb[:, tkc : tkc + 1],
                            scale=SCALE,
                        )
                        exs[(h, tkc)] = ex

                    def emit_pdav(h, tkc):
                        if tkc == 0:
                            pdavs[h] = (
                                pspd.tile([P, TQ], f32, tag="pd", name="pd"),
                                psum.tile([P, TQ], f32, tag="mm", name="pav"),
                            )
                        pd, pav = pdavs[h]
                        co = trim(tkc)
                        ex = exs.pop((h, tkc))
                        nc.tensor.matmul(
                            pd[:, co:],
                            ones_sb[:],
                            ex[:, co:],
                            start=(tkc == 0),
                            stop=(tkc == nchunks - 1),
                            skip_group_check=True,
                        )
                        nc.tensor.matmul(
                            pav[:, co:],
                            vq[h // 4][:, tkc, (h % 4) * P : (h % 4 + 1) * P],
                            ex[:, co:],
                            start=(tkc == 0),
                            stop=(tkc == nchunks - 1),
                            skip_group_check=True,
                        )
                        if tkc == nchunks - 1:
                            rec = recp.tile([P, TQ], f32, tag="rec", name="rec")
                            with nc.allow_low_precision("softmax denom"):
                                nc.vector.reciprocal(rec[:], pd[:])
                            nc.vector.tensor_tensor(
                                attB[:, h, :], pav[:], rec[:], ALU.mult)
                            # hi/lo fp8 split of the attention output for the
                            # DoubleRow output projection; on the otherwise
                            # idle gpsimd engine (SBUF-only operands)
                            dsc = recp.tile([P, TQ], bf16, tag="dsc", name="dsc")
                            nc.gpsimd.tensor_copy(attH[:, h, :], attB[:, h, :])
                            nc.gpsimd.tensor_tensor(
                                dsc[:], attB[:, h, :], attH[:, h, :],
                                ALU.subtract)
                            nc.gpsimd.tensor_scalar_mul(
                                attL[:, h, :], dsc[:], 256.0)

                    LAG = 3
                    units = [
                        (h, tkc) for h in range(HG) for tkc in range(nchunks)]
                    for i, (h, tkc) in enumerate(units):
                        emit_scores(h, tkc)
                        if i >= LAG:
                            emit_pdav(*units[i - LAG])
                    for i in range(len(units) - LAG, len(units)):
                        emit_pdav(*units[i])

                # ---- output projection, hi/lo fp8 DoubleRow: po_main gets
                # attH x woH; po_mid gets attH x woL + attL x woH (both
                # 256x-scaled); lo*lo is negligible and dropped. nt-outer /
                # t-inner so each wo tile is loaded once, not once per t ----
                for nt in range(NHID):
                    whs, wls = [], []
                    for half in range(2):
                        wh_ = wop.tile([P, 4, TQ], f8, tag="wo", name="wh_")
                        nc.sync.dma_start(wh_[:], wo[0, nt, half, :, :, :])
                        wl_ = wop.tile([P, 4, TQ], f8, tag="wo", name="wl_")
                        nc.sync.dma_start(wl_[:], wo[1, nt, half, :, :, :])
                        whs.append(wh_)
                        wls.append(wl_)
                    for t in (1, 0):
                        attH, attL = atts[t]
                        for tqc in range(TQ // P):
                            pom = psum.tile([P, TQ], f32, tag="mm", name="pom")
                            pol = pssc.tile([P, TQ], f32, tag="psc", name="pol")
                            for j in range(HG // 2):
                                lhsH = attH[:, 2 * j : 2 * j + 2,
                                            tqc * P : (tqc + 1) * P]
                                lhsL = attL[:, 2 * j : 2 * j + 2,
                                            tqc * P : (tqc + 1) * P]
                                rhsH = whs[j // 2][
                                    :, 2 * (j % 2) : 2 * (j % 2) + 2, :]
                                rhsL = wls[j // 2][
                                    :, 2 * (j % 2) : 2 * (j % 2) + 2, :]
                                nc.tensor.matmul(
                                    pom[:], lhsH, rhsH,
                                    start=(j == 0), stop=(j == HG // 2 - 1),
                                    perf_mode=DR)
                                nc.tensor.matmul(
                                    pol[:], lhsH, rhsL,
                                    start=(j == 0), stop=False,
                                    perf_mode=DR)
                                nc.tensor.matmul(
                                    pol[:], lhsL, rhsH,
                                    start=False, stop=(j == HG // 2 - 1),
                                    perf_mode=DR)
                            mid_sb = outp.tile(
                                [P, TQ], bf16, tag="mid", name="mid_sb")
                            nc.scalar.mul(mid_sb[:], pol[:], 1.0 / 256.0)
                            ot = outp.tile(
                                [P, TQ], mybir.dt.float16, tag="osb", name="ot")
                            nc.vector.tensor_tensor(
                                ot[:], mid_sb[:], pom[:], ALU.add)
                            nc.sync.dma_start(out[t, nt, tqc, :, :], ot[:])

    nc.finalize()
    return nc


def _get_program():
    if "nc" not in _CACHED:
        _CACHED["nc"] = _build_program()
    return _CACHED["nc"]


def _host_prep(x, wq_a, q_norm_w, wq_b, wkv_a, kv_norm_w, wkv_b, wo,
               attention_mask, positions):
    """Build the 8 per-core input maps (bf16 weights, partition-major)."""
    import ml_dtypes

    f = np.float32
    bf = ml_dtypes.bfloat16

    def hilo(w):
        # fp8 e4m3 hi + (residual*256) lo pair; exact to ~1e-3 of w
        f8t = ml_dtypes.float8_e4m3
        w = np.ascontiguousarray(w).astype(f)
        hi = w.astype(f8t)
        lo = ((w - hi.astype(f)) * 256.0).astype(f8t)
        return np.ascontiguousarray(np.stack([hi, lo]))

    x = np.asarray(x, f)
    wq_a = np.asarray(wq_a, f)
    wkv_a = np.asarray(wkv_a, f)
    # norm weights folded into the up-projection rows
    wq_b3 = (np.asarray(wq_b, f) * np.asarray(q_norm_w, f)[:, None]) \
        .reshape(Q_RANK, H, D_QK)
    wkv_b3 = (np.asarray(wkv_b, f) * np.asarray(kv_norm_w, f)[:, None]) \
        .reshape(KV_RANK, H, D_NOPE + D_V)
    wo2 = np.asarray(wo, f)
    attention_mask = np.asarray(attention_mask)
    positions = np.asarray(positions)

    # A-q weights chunk-major, hi/lo packed per chunk:
    # wqac[m, p, hl, kk, col] = hilo(wq_a)[hl, kk*128+p, m*128+col]
    wqac = np.ascontiguousarray(
        hilo(wq_a.reshape(HIDK, P, QCH, P).transpose(2, 1, 0, 3))
        .transpose(1, 2, 0, 3, 4))

    # A-kv weights quart-major (as original)
    wkva_blk = hilo(
        wkv_a[:, :KV_RANK].reshape(4, KQ, P, 2, 256).transpose(3, 0, 2, 1, 4))
    wkvr_blk = hilo(
        wkv_a[:, KV_RANK:].reshape(4, KQ, P, 64).transpose(0, 2, 1, 3))

    inv_freq = 1.0 / (
        THETA ** (np.arange(0, D_ROPE, 2, dtype=np.float64) / D_ROPE))

    dk = np.arange(P)[:, None]
    u = np.arange(7 * P)[None, :]
    cmask_ = np.where(dk <= u - 3 * P, 0.0, NMASK).astype(f)
    onesd = np.ones((P, P), bf)

    per_batch = {}
    for b in range(B):
        # xT[hl, k2, p, j, t] = hilo(x.T)[hl, (2*k2+j)*128+p, t]
        xTb = np.ascontiguousarray(
            hilo(x[b].T.reshape(HIDK // 2, 2, P, T).transpose(0, 2, 1, 3)))
        ang = positions[b].astype(np.float64)[:, None] * inv_freq[None, :]
        cosT = np.cos(ang).astype(f).T  # [32, T]
        sinT = np.sin(ang).astype(f).T
        cos4_ = np.ascontiguousarray(np.tile(cosT, (4, 1))).astype(bf)
        sin4_ = np.ascontiguousarray(
            np.concatenate([-sinT, sinT, -sinT, sinT], axis=0)).astype(bf)
        kb = np.where(attention_mask[b] != 0, 0.0, NMASK).astype(f)
        kbias_ = np.ascontiguousarray(kb.reshape(TC, P).T)
        per_batch[b] = (xTb, cos4_, sin4_, kbias_)

    in_maps = []
    for c in range(8):
        b, g = c // 4, c % 4
        hs = slice(g * HG, (g + 1) * HG)
        xTb, cos4_, sin4_, kbias_ = per_batch[b]
        # local x slab: xloc[hl, p, kk, col] = xT[hl, kk*128+p, g*TL+col]
        xloc_ = hilo(
            x[b][g * TL : (g + 1) * TL, :].T
            .reshape(HIDK, P, TL).transpose(1, 0, 2))

        wqbn_ = hilo(
            wq_b3[:, hs, :D_NOPE]
            .reshape(QCH, P, HG, P).transpose(2, 1, 0, 3))
        # rope cols packed in head pairs: [h_even 64 | h_odd 64] per 128-col
        wqbr_ = hilo(
            wq_b3[:, hs, D_NOPE:]
            .reshape(QCH, P, HG // 2, P).transpose(2, 1, 0, 3))
        wkbn_ = np.ascontiguousarray(
            wkv_b3[:, hs, :D_NOPE]
            .reshape(KCH, P, HG, P).transpose(1, 2, 0, 3)).astype(bf)
        wkbv_ = np.ascontiguousarray(
            wkv_b3[:, hs, D_NOPE:]
            .reshape(KCH, P, 2, TQ).transpose(2, 1, 0, 3)).astype(bf)
        wosh = hilo(
            wo2[g * HG * D_V : (g + 1) * HG * D_V, :]
            .reshape(2, 4, P, NHID, TQ).transpose(3, 0, 2, 1, 4))
        in_maps.append({
            "xloc": xloc_, "xT": xTb, "wqac": wqac,
            "wkva": wkva_blk, "wkvr": wkvr_blk,
            "wqbn": wqbn_, "wqbr": wqbr_, "wkbn": wkbn_, "wkbv": wkbv_,
            "wo": wosh,
            "cos4": cos4_, "sin4": sin4_,
            "cmask": cmask_, "kbias": kbias_, "onesd": onesd,
        })
    return in_maps


def kernel(**inputs):
    from concourse.bass_utils import run_bass_kernel_spmd

    nc = _get_program()
    in_maps = _host_prep(**inputs)
    res = run_bass_kernel_spmd(nc, in_maps, core_ids=list(range(8)))
    _CACHED["last_result"] = res
    out = np.zeros((B, T, HID), np.float32)
    for c in range(8):
        blk = np.asarray(res.results[c]["out"], np.float32)
        # row = t*512 + q*128 + p, col = nt*512 + c
        out[c // 4] += blk.transpose(0, 2, 3, 1, 4).reshape(T, HID)
    return out
